# revision 20
# baseline (speedup 1.0000x reference)
"""
w4a8 fake-quant linear for Trainium2, 8-core SPMD.

  y[b,s,o] = x_dq[b,s,:] . w_dq[o,:]
    x_dq: per-token int8 fake quant-dequant of x
    w_dq: per-channel-group dequant of int4 weights

Sharding: tokens (B*S = 16384) split across the 8 cores; each core computes
its [2048, 2048] output slice against the full weight matrix (compute-bound;
weight/output sharding would force every core to re-read all of x and be
HBM-bound instead).

Host prep: weights are dequantized to bf16 and pre-transposed to [I, O]
(one-time O(N^2) repack; values are int4-grid * group scale, bf16 rounding
is ~2^-9 relative on the weight side only).

Device math: per-token quant produces n = clip(round(x/s)+zp) - zp, which is
an *integer* in [-255, 255] -- exactly representable in bf16.  The matmul
accumulates in fp32 PSUM, and the per-token scale s is applied on PSUM
eviction.  round() is jnp-compatible RNE via the magic-number trick.
"""

import os

import numpy as np
import ml_dtypes

import concourse.bass as bass
import concourse.mybir as mybir
import concourse.tile as tile
from concourse.bass_utils import run_bass_kernel_spmd
from concourse.masks import make_identity
from concourse.tile import add_dep_helper


def _raw(inst):
    return getattr(inst, "ins", inst)


def _legalize_waits(nc):
    """Split multi-wait instructions for this walrus build.

    The neuronxcc walrus here supports exactly ONE sync wait per TPB
    instruction (setupSyncWait raises "Too many sync wait commands"
    otherwise).  Tile emits up to ~3 waits per instruction.  Every engine
    executes its instruction stream in order, so hoisting the extra waits
    into standalone EVENT_SEMAPHORE instructions placed immediately before
    the instruction (on the same engine) is semantically identical.
    """
    import bass_rust

    fn = nc.m.functions[0]
    ctr = 0
    new_blocks = []
    for b in fn.blocks:
        out = []
        for i in b.instructions:
            si = i.sync_info
            if si is not None and len(si.on_wait) > 1:
                waits = list(si.on_wait)
                # For DMAs keep the own-lane (ring pacing) wait attached if
                # present; otherwise keep the last one.  All other waits
                # become standalone event-sem stalls just before it.
                own = {u.ant_name for u in si.on_update}
                keep_idx = len(waits) - 1
                for k, w in enumerate(waits):
                    if w.ant_name in own:
                        keep_idx = k
                        break
                for k, w in enumerate(waits):
                    if k == keep_idx:
                        continue
                    ctr += 1
                    es = mybir.InstEventSemaphore(name=f"I-eswait{ctr}")
                    es.engine = i.engine
                    es.sync_info = mybir.SyncInfo(on_wait=[w], on_update=[])
                    out.append(es)
                si.on_wait = [waits[keep_idx]]
            out.append(i)
        new_blocks.append(bass_rust.BasicBlock(name=b.name, instructions=out))
    fn.blocks = new_blocks

NCORES = 8
B, S, I, O = 4, 4096, 2048, 2048
GROUP = 32
TOK = B * S            # 16384 tokens
TPC = TOK // NCORES    # 2048 tokens per core
P = 128
TT = TPC // P          # 16 token tiles per core
KK = I // P            # 16 contraction chunks
NBANK = 512            # fp32 PSUM bank width
NJ = O // NBANK        # 4 psum banks per token tile

MAGIC = 12582912.0     # 1.5 * 2**23: RNE round for |v| < 2**22
EPS = float(np.finfo(np.float32).eps)

_cached_nc = None
last_results = None    # for test harness introspection (exec_time_ns etc.)


def _build_nc():
    nc = bass.Bass()
    f32 = mybir.dt.float32
    bf16 = mybir.dt.bfloat16
    X = mybir.AxisListType.X
    A = mybir.AluOpType

    # Per-token-tile DRAM tensors: Tile tracks DRAM conflicts at tensor
    # granularity, so a single x/y tensor would chain every load/store DMA
    # into a WAW/WAR sequence (and DIRECT2D DMAs only support one sync wait).
    xs = [
        nc.declare_dram_parameter(f"x{t:02d}", [P, I], f32, isOutput=False)
        for t in range(TT)
    ]
    wt = nc.declare_dram_parameter("wt", [I, O], bf16, isOutput=False)
    ys = [
        nc.declare_dram_parameter(f"y{t:02d}", [P, O], f32, isOutput=True)
        for t in range(TT)
    ]

    with tile.TileContext(nc) as tc:
        # HWDGE dynamic-DMA instructions only support <=2 sync waits, so the
        # pool-recycled streams (x loads, y stores) go through gpsimd SWDGE.
        # The n -> nt transpose runs on the tensor engine (a SBUF->SBUF DMA
        # transpose would xbar-serialize against every in-flight DMA).
        with (
            tc.tile_pool(name="wpool", bufs=1) as wpool,
            tc.tile_pool(name="consts", bufs=1) as consts,
            tc.tile_pool(name="xpool", bufs=4) as xpool,
            tc.tile_pool(name="npool", bufs=3) as npool,
            tc.tile_pool(name="ntpool", bufs=3) as ntpool,
            tc.tile_pool(name="ypool", bufs=2) as ypool,
            tc.tile_pool(name="small", bufs=6) as small,
            tc.tile_pool(name="psum_y", bufs=1, space="PSUM") as psum_y,
            tc.tile_pool(name="psum_t", bufs=4, space="PSUM") as psum_t,
        ):
            identity = consts.tile([P, P], bf16)
            make_identity(nc, identity)

            # resident transposed weights: wt_sb[p, kk, o] = w_dq[o, kk*128+p]
            wt_sb = wpool.tile([P, KK, O], bf16)
            wt_r = wt.rearrange("(kk p) o -> p kk o", p=P)
            for kk in range(KK):
                nc.sync.dma_start(out=wt_sb[:, kk, :], in_=wt_r[:, kk, :])

            for tt in range(TT):
                x_t = xpool.tile([P, I], f32)
                nc.gpsimd.dma_start(out=x_t, in_=xs[tt][:, :])

                mx = small.tile([P, 1], f32, tag="mx")
                mn = small.tile([P, 1], f32, tag="mn")
                nc.vector.tensor_reduce(mx, x_t, X, A.max)
                nc.vector.tensor_reduce(mn, x_t, X, A.min)
                nc.vector.tensor_scalar(mx, mx, 0.0, None, A.max)
                nc.vector.tensor_scalar(mn, mn, 0.0, None, A.min)
                # s = max((mx - mn)/255, eps); inv = 1/s
                # (DVE has no divide ALU op; *1/255 differs by <=1 ulp)
                s = small.tile([P, 1], f32, tag="s")
                nc.vector.tensor_tensor(s, mx, mn, A.subtract)
                nc.vector.tensor_scalar(s, s, 1.0 / 255.0, EPS, A.mult, A.max)
                inv = small.tile([P, 1], f32, tag="inv")
                nc.vector.reciprocal(inv, s)
                # hi = 127 - zp = 255 + round(mn * inv)
                hi = small.tile([P, 1], f32, tag="hi")
                nc.vector.tensor_tensor(hi, mn, inv, A.mult)
                nc.vector.tensor_scalar(hi, hi, MAGIC, None, A.add)
                nc.vector.tensor_scalar(hi, hi, MAGIC, 255.0, A.subtract, A.add)
                # n = min(round(x*inv), hi)  (lower clip provably inactive)
                q = npool.tile([P, I], f32, tag="q")
                nc.vector.tensor_scalar(q, x_t, inv, MAGIC, A.mult, A.add)
                n_bf = npool.tile([P, I], bf16, tag="n")
                nc.vector.tensor_scalar(n_bf, q, MAGIC, hi, A.subtract, A.min)

                # nt[p, kk, t] = n[t, kk*128 + p] via PE transpose
                nt = ntpool.tile([P, KK, P], bf16)
                ypsum = psum_y.tile([P, O], f32)
                for kk in range(KK):
                    pt = psum_t.tile([P, P], bf16)
                    nc.tensor.transpose(pt, n_bf[:, kk * P:(kk + 1) * P], identity)
                    nc.scalar.copy(nt[:, kk, :], pt)
                    for j in range(NJ):
                        nc.tensor.matmul(
                            ypsum[:, j * NBANK:(j + 1) * NBANK],
                            lhsT=nt[:, kk, :],
                            rhs=wt_sb[:, kk, j * NBANK:(j + 1) * NBANK],
                            start=(kk == 0),
                            stop=(kk == KK - 1),
                        )

                # evict on DVE: s lives on DVE, so this stays at 2 sem waits
                y_sb = ypool.tile([P, O], f32)
                for j in range(NJ):
                    nc.vector.tensor_scalar_mul(
                        y_sb[:, j * NBANK:(j + 1) * NBANK],
                        ypsum[:, j * NBANK:(j + 1) * NBANK],
                        s,
                    )
                nc.gpsimd.dma_start(out=ys[tt][:, :], in_=y_sb)

    _legalize_waits(nc)
    return nc


def kernel(x, w_q, w_scales, w_zeros):
    global _cached_nc, last_results
    if _cached_nc is None:
        _cached_nc = _build_nc()
    nc = _cached_nc

    x2 = np.ascontiguousarray(np.asarray(x, dtype=np.float32).reshape(TOK, I))
    s_e = np.repeat(np.asarray(w_scales, dtype=np.float32), GROUP, axis=1)
    z_e = np.repeat(np.asarray(w_zeros, dtype=np.float32), GROUP, axis=1)
    w_dq = (np.asarray(w_q).astype(np.float32) - z_e) * s_e
    wt = np.ascontiguousarray(w_dq.T).astype(ml_dtypes.bfloat16)

    in_maps = []
    for c in range(NCORES):
        m = {"wt": wt}
        for t in range(TT):
            base = c * TPC + t * P
            m[f"x{t:02d}"] = x2[base:base + P]
        in_maps.append(m)
    trace = os.environ.get("BASS_KERNEL_TRACE") == "1"
    res = run_bass_kernel_spmd(nc, in_maps, list(range(NCORES)), trace=trace)
    last_results = res
    out = np.concatenate(
        [res.results[c][f"y{t:02d}"] for c in range(NCORES) for t in range(TT)],
        axis=0,
    )
    return np.ascontiguousarray(out.reshape(B, S, O).astype(np.float32))


# revision 22
# speedup vs baseline: 1.0230x; 1.0230x over previous
"""
w4a8 fake-quant linear for Trainium2, 8-core SPMD.

  y[b,s,o] = x_dq[b,s,:] . w_dq[o,:]
    x_dq: per-token int8 fake quant-dequant of x
    w_dq: per-channel-group dequant of int4 weights

Sharding: tokens (B*S = 16384) split across the 8 cores; each core computes
its [2048, 2048] output slice against the full weight matrix (compute-bound;
weight/output sharding would force every core to re-read all of x and be
HBM-bound instead).

Host prep: weights are dequantized to bf16 and pre-transposed to [I, O]
(one-time O(N^2) repack; values are int4-grid * group scale, bf16 rounding
is ~2^-9 relative on the weight side only).

Device math: per-token quant produces n = clip(round(x/s)+zp) - zp, which is
an *integer* in [-255, 255] -- exactly representable in bf16.  The matmul
accumulates in fp32 PSUM, and the per-token scale s is applied on PSUM
eviction.  round() is jnp-compatible RNE via the magic-number trick.
"""

import os

import numpy as np
import ml_dtypes

import concourse.bass as bass
import concourse.mybir as mybir
import concourse.tile as tile
from concourse.bass_utils import run_bass_kernel_spmd
from concourse.masks import make_identity
from concourse.tile import add_dep_helper


def _raw(inst):
    return getattr(inst, "ins", inst)


def _legalize_waits(nc):
    """Split multi-wait instructions for this walrus build.

    The neuronxcc walrus here supports exactly ONE sync wait per TPB
    instruction (setupSyncWait raises "Too many sync wait commands"
    otherwise).  Tile emits up to ~3 waits per instruction.  Every engine
    executes its instruction stream in order, so hoisting the extra waits
    into standalone EVENT_SEMAPHORE instructions placed immediately before
    the instruction (on the same engine) is semantically identical.
    """
    import bass_rust

    fn = nc.m.functions[0]
    ctr = 0
    new_blocks = []
    for b in fn.blocks:
        out = []
        for i in b.instructions:
            si = i.sync_info
            if si is not None and len(si.on_wait) > 1:
                waits = list(si.on_wait)
                # For DMAs keep the own-lane (ring pacing) wait attached if
                # present; otherwise keep the last one.  All other waits
                # become standalone event-sem stalls just before it.
                own = {u.ant_name for u in si.on_update}
                keep_idx = len(waits) - 1
                for k, w in enumerate(waits):
                    if w.ant_name in own:
                        keep_idx = k
                        break
                for k, w in enumerate(waits):
                    if k == keep_idx:
                        continue
                    ctr += 1
                    es = mybir.InstEventSemaphore(name=f"I-eswait{ctr}")
                    es.engine = i.engine
                    es.sync_info = mybir.SyncInfo(on_wait=[w], on_update=[])
                    out.append(es)
                si.on_wait = [waits[keep_idx]]
            out.append(i)
        new_blocks.append(bass_rust.BasicBlock(name=b.name, instructions=out))
    fn.blocks = new_blocks

NCORES = 8
B, S, I, O = 4, 4096, 2048, 2048
GROUP = 32
TOK = B * S            # 16384 tokens
TPC = TOK // NCORES    # 2048 tokens per core
P = 128
TT = TPC // P          # 16 token tiles per core
KK = I // P            # 16 contraction chunks
NBANK = 512            # fp32 PSUM bank width
NJ = O // NBANK        # 4 psum banks per token tile

MAGIC = 12582912.0     # 1.5 * 2**23: RNE round for |v| < 2**22
EPS = float(np.finfo(np.float32).eps)

_cached_nc = None
last_results = None    # for test harness introspection (exec_time_ns etc.)


def _build_nc():
    nc = bass.Bass()
    f32 = mybir.dt.float32
    bf16 = mybir.dt.bfloat16
    X = mybir.AxisListType.X
    A = mybir.AluOpType

    # Per-token-tile DRAM tensors: Tile tracks DRAM conflicts at tensor
    # granularity, so a single x/y tensor would chain every load/store DMA
    # into a WAW/WAR sequence (and DIRECT2D DMAs only support one sync wait).
    xs = [
        nc.declare_dram_parameter(f"x{t:02d}", [P, I], f32, isOutput=False)
        for t in range(TT)
    ]
    wt = nc.declare_dram_parameter("wt", [I, O], bf16, isOutput=False)
    ys = [
        nc.declare_dram_parameter(f"y{t:02d}", [P, O], f32, isOutput=True)
        for t in range(TT)
    ]

    with tile.TileContext(nc) as tc:
        # HWDGE dynamic-DMA instructions only support <=2 sync waits, so the
        # pool-recycled streams (x loads, y stores) go through gpsimd SWDGE.
        # The n -> nt transpose runs on the tensor engine (a SBUF->SBUF DMA
        # transpose would xbar-serialize against every in-flight DMA).
        with (
            tc.tile_pool(name="wpool", bufs=1) as wpool,
            tc.tile_pool(name="consts", bufs=1) as consts,
            tc.tile_pool(name="xpool", bufs=4) as xpool,
            tc.tile_pool(name="npool", bufs=3) as npool,
            tc.tile_pool(name="ntpool", bufs=3) as ntpool,
            tc.tile_pool(name="ypool", bufs=2) as ypool,
            tc.tile_pool(name="small", bufs=6) as small,
            tc.tile_pool(name="psum_y", bufs=1, space="PSUM") as psum_y,
            tc.tile_pool(name="psum_t", bufs=4, space="PSUM") as psum_t,
        ):
            identity = consts.tile([P, P], bf16)
            make_identity(nc, identity)

            # Issue the first x loads before the weight stream so the quant
            # pipeline (DVE) and first transposes start while wt streams in.
            x_tiles = {}
            for tt in range(min(3, TT)):
                x_t = xpool.tile([P, I], f32)
                nc.gpsimd.dma_start(out=x_t, in_=xs[tt][:, :])
                x_tiles[tt] = x_t

            # resident transposed weights: wt_sb[p, kk, o] = w_dq[o, kk*128+p]
            wt_sb = wpool.tile([P, KK, O], bf16)
            wt_r = wt.rearrange("(kk p) o -> p kk o", p=P)
            for kk in range(KK):
                nc.sync.dma_start(out=wt_sb[:, kk, :], in_=wt_r[:, kk, :])

            for tt in range(TT):
                if tt in x_tiles:
                    x_t = x_tiles[tt]
                else:
                    x_t = xpool.tile([P, I], f32)
                    nc.gpsimd.dma_start(out=x_t, in_=xs[tt][:, :])

                mx = small.tile([P, 1], f32, tag="mx")
                mn = small.tile([P, 1], f32, tag="mn")
                nc.vector.tensor_reduce(mx, x_t, X, A.max)
                nc.vector.tensor_reduce(mn, x_t, X, A.min)
                nc.vector.tensor_scalar(mx, mx, 0.0, None, A.max)
                nc.vector.tensor_scalar(mn, mn, 0.0, None, A.min)
                # s = max((mx - mn)/255, eps); inv = 1/s
                # (DVE has no divide ALU op; *1/255 differs by <=1 ulp)
                s = small.tile([P, 1], f32, tag="s")
                nc.vector.tensor_tensor(s, mx, mn, A.subtract)
                nc.vector.tensor_scalar(s, s, 1.0 / 255.0, EPS, A.mult, A.max)
                inv = small.tile([P, 1], f32, tag="inv")
                nc.vector.reciprocal(inv, s)
                # hi = 127 - zp = 255 + round(mn * inv)
                hi = small.tile([P, 1], f32, tag="hi")
                nc.vector.tensor_tensor(hi, mn, inv, A.mult)
                nc.vector.tensor_scalar(hi, hi, MAGIC, None, A.add)
                nc.vector.tensor_scalar(hi, hi, MAGIC, 255.0, A.subtract, A.add)
                # n = min(round(x*inv), hi)  (lower clip provably inactive)
                q = npool.tile([P, I], f32, tag="q")
                nc.vector.tensor_scalar(q, x_t, inv, MAGIC, A.mult, A.add)
                n_bf = npool.tile([P, I], bf16, tag="n")
                nc.vector.tensor_scalar(n_bf, q, MAGIC, hi, A.subtract, A.min)

                # nt[p, kk, t] = n[t, kk*128 + p] via PE transpose.  All 16
                # transposes are emitted before the matmul block so the
                # PE stream for tile tt+1's transposes overlaps the ACT
                # copybacks while tile tt's matmuls still run.
                nt = ntpool.tile([P, KK, P], bf16)
                for kk in range(KK):
                    pt = psum_t.tile([P, P], bf16)
                    nc.tensor.transpose(pt, n_bf[:, kk * P:(kk + 1) * P], identity)
                    nc.scalar.copy(nt[:, kk, :], pt)

                ypsum = psum_y.tile([P, O], f32)
                for kk in range(KK):
                    for j in range(NJ):
                        nc.tensor.matmul(
                            ypsum[:, j * NBANK:(j + 1) * NBANK],
                            lhsT=nt[:, kk, :],
                            rhs=wt_sb[:, kk, j * NBANK:(j + 1) * NBANK],
                            start=(kk == 0),
                            stop=(kk == KK - 1),
                        )

                # evict on DVE: s lives on DVE, so this stays at 2 sem waits
                y_sb = ypool.tile([P, O], f32)
                for j in range(NJ):
                    nc.vector.tensor_scalar_mul(
                        y_sb[:, j * NBANK:(j + 1) * NBANK],
                        ypsum[:, j * NBANK:(j + 1) * NBANK],
                        s,
                    )
                nc.gpsimd.dma_start(out=ys[tt][:, :], in_=y_sb)

    _legalize_waits(nc)
    return nc


def kernel(x, w_q, w_scales, w_zeros):
    global _cached_nc, last_results
    if _cached_nc is None:
        _cached_nc = _build_nc()
    nc = _cached_nc

    x2 = np.ascontiguousarray(np.asarray(x, dtype=np.float32).reshape(TOK, I))
    s_e = np.repeat(np.asarray(w_scales, dtype=np.float32), GROUP, axis=1)
    z_e = np.repeat(np.asarray(w_zeros, dtype=np.float32), GROUP, axis=1)
    w_dq = (np.asarray(w_q).astype(np.float32) - z_e) * s_e
    wt = np.ascontiguousarray(w_dq.T).astype(ml_dtypes.bfloat16)

    in_maps = []
    for c in range(NCORES):
        m = {"wt": wt}
        for t in range(TT):
            base = c * TPC + t * P
            m[f"x{t:02d}"] = x2[base:base + P]
        in_maps.append(m)
    trace = os.environ.get("BASS_KERNEL_TRACE") == "1"
    res = run_bass_kernel_spmd(nc, in_maps, list(range(NCORES)), trace=trace)
    last_results = res
    out = np.concatenate(
        [res.results[c][f"y{t:02d}"] for c in range(NCORES) for t in range(TT)],
        axis=0,
    )
    return np.ascontiguousarray(out.reshape(B, S, O).astype(np.float32))


# revision 24
# speedup vs baseline: 1.1898x; 1.1630x over previous
"""
w4a8 fake-quant linear for Trainium2, 8-core SPMD.

  y[b,s,o] = x_dq[b,s,:] . w_dq[o,:]
    x_dq: per-token int8 fake quant-dequant of x
    w_dq: per-channel-group dequant of int4 weights

Sharding: tokens (B*S = 16384) split across the 8 cores; each core computes
its [2048, 2048] output slice against the full weight matrix (compute-bound;
weight/output sharding would force every core to re-read all of x and be
HBM-bound instead).

Host prep: weights are dequantized to bf16 and pre-transposed to [I, O]
(one-time O(N^2) repack; values are int4-grid * group scale, bf16 rounding
is ~2^-9 relative on the weight side only).

Device math: per-token quant produces n = clip(round(x/s)+zp) - zp, which is
an *integer* in [-255, 255] -- exactly representable in bf16.  The matmul
accumulates in fp32 PSUM, and the per-token scale s is applied on PSUM
eviction.  round() is jnp-compatible RNE via the magic-number trick.
"""

import os

import numpy as np
import ml_dtypes

import concourse.bass as bass
import concourse.mybir as mybir
import concourse.tile as tile
from concourse.bass_utils import run_bass_kernel_spmd
from concourse.masks import make_identity
from concourse.tile import add_dep_helper


def _raw(inst):
    return getattr(inst, "ins", inst)


def _legalize_waits(nc):
    """Split multi-wait instructions for this walrus build.

    The neuronxcc walrus here supports exactly ONE sync wait per TPB
    instruction (setupSyncWait raises "Too many sync wait commands"
    otherwise).  Tile emits up to ~3 waits per instruction.  Every engine
    executes its instruction stream in order, so hoisting the extra waits
    into standalone EVENT_SEMAPHORE instructions placed immediately before
    the instruction (on the same engine) is semantically identical.
    """
    import bass_rust

    fn = nc.m.functions[0]
    ctr = 0
    new_blocks = []
    for b in fn.blocks:
        out = []
        for i in b.instructions:
            si = i.sync_info
            if si is not None and len(si.on_wait) > 1:
                waits = list(si.on_wait)
                # For DMAs keep the own-lane (ring pacing) wait attached if
                # present; otherwise keep the last one.  All other waits
                # become standalone event-sem stalls just before it.
                own = {u.ant_name for u in si.on_update}
                keep_idx = len(waits) - 1
                for k, w in enumerate(waits):
                    if w.ant_name in own:
                        keep_idx = k
                        break
                for k, w in enumerate(waits):
                    if k == keep_idx:
                        continue
                    ctr += 1
                    es = mybir.InstEventSemaphore(name=f"I-eswait{ctr}")
                    es.engine = i.engine
                    es.sync_info = mybir.SyncInfo(on_wait=[w], on_update=[])
                    out.append(es)
                si.on_wait = [waits[keep_idx]]
            out.append(i)
        new_blocks.append(bass_rust.BasicBlock(name=b.name, instructions=out))
    fn.blocks = new_blocks

NCORES = 8
B, S, I, O = 4, 4096, 2048, 2048
GROUP = 32
TOK = B * S            # 16384 tokens
TPC = TOK // NCORES    # 2048 tokens per core
P = 128
TT = TPC // P          # 16 token tiles per core
KK = I // P            # 16 contraction chunks
NBANK = 512            # fp32 PSUM bank width
NJ = O // NBANK        # 4 psum banks per token tile

MAGIC = 12582912.0     # 1.5 * 2**23: RNE round for |v| < 2**22
EPS = float(np.finfo(np.float32).eps)

_cached_nc = None
last_results = None    # for test harness introspection (exec_time_ns etc.)


def _build_nc():
    nc = bass.Bass()
    f32 = mybir.dt.float32
    bf16 = mybir.dt.bfloat16
    X = mybir.AxisListType.X
    A = mybir.AluOpType

    # Per-token-tile DRAM tensors: Tile tracks DRAM conflicts at tensor
    # granularity, so a single x/y tensor would chain every load/store DMA
    # into a WAW/WAR sequence (and DIRECT2D DMAs only support one sync wait).
    xs = [
        nc.declare_dram_parameter(f"x{t:02d}", [P, I], f32, isOutput=False)
        for t in range(TT)
    ]
    wt = nc.declare_dram_parameter("wt", [I, O], bf16, isOutput=False)
    ys = [
        nc.declare_dram_parameter(f"y{t:02d}", [P, O], f32, isOutput=True)
        for t in range(TT)
    ]

    with tile.TileContext(nc) as tc:
        # HWDGE dynamic-DMA instructions only support <=2 sync waits, so the
        # pool-recycled streams (x loads, y stores) go through gpsimd SWDGE.
        # The n -> nt transpose runs on the tensor engine (a SBUF->SBUF DMA
        # transpose would xbar-serialize against every in-flight DMA).
        with (
            tc.tile_pool(name="wpool", bufs=1) as wpool,
            tc.tile_pool(name="consts", bufs=1) as consts,
            tc.tile_pool(name="xpool", bufs=4) as xpool,
            tc.tile_pool(name="npool", bufs=3) as npool,
            tc.tile_pool(name="ntpool", bufs=3) as ntpool,
            tc.tile_pool(name="ypool", bufs=2) as ypool,
            tc.tile_pool(name="small", bufs=6) as small,
            tc.tile_pool(name="psum_y", bufs=2, space="PSUM") as psum_y,
        ):
            # Issue the first x loads before the weight stream so the quant
            # pipeline (DVE) and first transposes start while wt streams in.
            x_tiles = {}
            for tt in range(min(3, TT)):
                x_t = xpool.tile([P, I], f32)
                nc.gpsimd.dma_start(out=x_t, in_=xs[tt][:, :])
                x_tiles[tt] = x_t

            # resident transposed weights: wt_sb[p, kk, o] = w_dq[o, kk*128+p]
            wt_sb = wpool.tile([P, KK, O], bf16)
            wt_r = wt.rearrange("(kk p) o -> p kk o", p=P)
            for kk in range(KK):
                nc.sync.dma_start(out=wt_sb[:, kk, :], in_=wt_r[:, kk, :])

            for tt in range(TT):
                if tt in x_tiles:
                    x_t = x_tiles[tt]
                else:
                    x_t = xpool.tile([P, I], f32)
                    nc.gpsimd.dma_start(out=x_t, in_=xs[tt][:, :])

                mx = small.tile([P, 1], f32, tag="mx")
                mn = small.tile([P, 1], f32, tag="mn")
                nc.vector.tensor_reduce(mx, x_t, X, A.max)
                nc.vector.tensor_reduce(mn, x_t, X, A.min)
                nc.vector.tensor_scalar(mx, mx, 0.0, None, A.max)
                nc.vector.tensor_scalar(mn, mn, 0.0, None, A.min)
                # s = max((mx - mn)/255, eps); inv = 1/s
                # (DVE has no divide ALU op; *1/255 differs by <=1 ulp)
                s = small.tile([P, 1], f32, tag="s")
                nc.vector.tensor_tensor(s, mx, mn, A.subtract)
                nc.vector.tensor_scalar(s, s, 1.0 / 255.0, EPS, A.mult, A.max)
                inv = small.tile([P, 1], f32, tag="inv")
                nc.vector.reciprocal(inv, s)
                # hi = 127 - zp = 255 + round(mn * inv)
                hi = small.tile([P, 1], f32, tag="hi")
                nc.vector.tensor_tensor(hi, mn, inv, A.mult)
                nc.vector.tensor_scalar(hi, hi, MAGIC, None, A.add)
                nc.vector.tensor_scalar(hi, hi, MAGIC, 255.0, A.subtract, A.add)
                # n = min(round(x*inv), hi)  (lower clip provably inactive)
                q = npool.tile([P, I], f32, tag="q")
                nc.vector.tensor_scalar(q, x_t, inv, MAGIC, A.mult, A.add)
                n_bf = npool.tile([P, I], bf16, tag="n")
                nc.vector.tensor_scalar(n_bf, q, MAGIC, hi, A.subtract, A.min)

                # nt[p, kk, t] = n[t, kk*128 + p] via SBUF->SBUF DMA
                # transpose (keeps the tensor engine free for matmuls)
                nt = ntpool.tile([P, KK, P], bf16)
                nc.sync.dma_start_transpose(nt, n_bf)

                ypsum = psum_y.tile([P, O], f32)
                for kk in range(KK):
                    for j in range(NJ):
                        nc.tensor.matmul(
                            ypsum[:, j * NBANK:(j + 1) * NBANK],
                            lhsT=nt[:, kk, :],
                            rhs=wt_sb[:, kk, j * NBANK:(j + 1) * NBANK],
                            start=(kk == 0),
                            stop=(kk == KK - 1),
                        )

                # evict on DVE: s lives on DVE, so this stays at 2 sem waits
                y_sb = ypool.tile([P, O], f32)
                for j in range(NJ):
                    nc.vector.tensor_scalar_mul(
                        y_sb[:, j * NBANK:(j + 1) * NBANK],
                        ypsum[:, j * NBANK:(j + 1) * NBANK],
                        s,
                    )
                nc.gpsimd.dma_start(out=ys[tt][:, :], in_=y_sb)

    _legalize_waits(nc)
    return nc


def kernel(x, w_q, w_scales, w_zeros):
    global _cached_nc, last_results
    if _cached_nc is None:
        _cached_nc = _build_nc()
    nc = _cached_nc

    x2 = np.ascontiguousarray(np.asarray(x, dtype=np.float32).reshape(TOK, I))
    s_e = np.repeat(np.asarray(w_scales, dtype=np.float32), GROUP, axis=1)
    z_e = np.repeat(np.asarray(w_zeros, dtype=np.float32), GROUP, axis=1)
    w_dq = (np.asarray(w_q).astype(np.float32) - z_e) * s_e
    wt = np.ascontiguousarray(w_dq.T).astype(ml_dtypes.bfloat16)

    in_maps = []
    for c in range(NCORES):
        m = {"wt": wt}
        for t in range(TT):
            base = c * TPC + t * P
            m[f"x{t:02d}"] = x2[base:base + P]
        in_maps.append(m)
    trace = os.environ.get("BASS_KERNEL_TRACE") == "1"
    res = run_bass_kernel_spmd(nc, in_maps, list(range(NCORES)), trace=trace)
    last_results = res
    out = np.concatenate(
        [res.results[c][f"y{t:02d}"] for c in range(NCORES) for t in range(TT)],
        axis=0,
    )
    return np.ascontiguousarray(out.reshape(B, S, O).astype(np.float32))
